# revision 2
# baseline (speedup 1.0000x reference)
"""NetVLAD Trainium2 kernel — data-parallel over N across 8 cores.

Per core: 4 images [C=128, P=4096].  Pipeline per 1024-pixel chunk:
  PE:   logits[p,k] = x_tile.T @ conv_wT   (x_tile stationary, shared with
        the x-transpose matmul x_tile.T @ I -> xT[p,c]); ssq[p] via
        xsq_tile.T @ ones.
  DVE/POOL/ACT: softmax over k in [pixel-partition, k-free] layout with
        per-pixel scalars held as [128, 8] stat columns and broadcast via
        step-0 access patterns.
  PE:   vlad^[k,c] += a_r.T-style accumulation: lhsT=a_r[:, :56],
        rhs=xT -> psum[56,128]; cluster mass s_k via rhs=n-col.
Final per image: vlad = term1 - s*cen, intra-normalize over k (via PE
transpose), global normalize, write [56,128] rows.
"""

import os
import sys

for _p in ("/opt/trn_rl_repo",):
    if _p not in sys.path:
        sys.path.insert(0, _p)

import numpy as np

NIMG = 4      # images per core
C = 128
K = 64
KE = 56
P = 4096
TPC = 8       # pixel tiles (128 px) per chunk
CH = TPC * 128
NCH = P // CH  # 4 chunks per image

_cache = {}


def _build():
    import concourse.bass as bass
    import concourse.mybir as mybir
    from concourse import bacc, tile

    f32 = mybir.dt.float32
    Alu = mybir.AluOpType
    Act = mybir.ActivationFunctionType

    nc = bacc.Bacc()
    x_in = nc.declare_dram_parameter("x", [NIMG, C, P], f32, isOutput=False)
    # packed consts: wT[0:64] | b8[64:576] | ident[576:704] | ones[704:832]
    # | cen[832:960] (partitions 0:56)
    cst_in = nc.declare_dram_parameter("consts", [C, 960], f32, isOutput=False)
    out_ext = nc.declare_dram_parameter("out", [NIMG, KE, C], f32, isOutput=True)
    dbg_ext = nc.declare_dram_parameter("dbg", [C, 680], f32, isOutput=True)

    with tile.TileContext(nc) as tc:
        with (
            tc.tile_pool(name="const", bufs=1) as cpool,
            tc.tile_pool(name="xin", bufs=3) as xpool,
            tc.tile_pool(name="work", bufs=2) as wpool,
            tc.tile_pool(name="stats", bufs=2) as spool,
            tc.tile_pool(name="fin", bufs=2) as fpool,
            tc.tile_pool(name="psL", bufs=2, space="PSUM") as pL,
            tc.tile_pool(name="psT", bufs=1, space="PSUM") as pT,
            tc.tile_pool(name="psS", bufs=2, space="PSUM") as pS,
            tc.tile_pool(name="psV", bufs=1, space="PSUM") as pV,
            tc.tile_pool(name="psF", bufs=1, space="PSUM") as pF,
        ):
            cst = cpool.tile([C, 960], f32, tag="cst")
            nc.gpsimd.dma_start(cst[:], cst_in[:])
            wT = cst[:, 0:K]
            b8 = cst[:, 64:64 + TPC * K]
            ident = cst[:, 576:576 + C]
            onesc = cst[:, 704:705]
            onesr = cst[0:1, 704:704 + C]
            cen = cst[0:KE, 832:832 + C]

            # PE warm-up: make PE observe the const-DMA semaphore once, so
            # later matmuls need at most one additional wait each.
            warm = pL.tile([C, TPC * K], f32, tag="L")
            nc.tensor.matmul(warm[0:1, 0:1], onesc, onesc,
                             start=True, stop=True)

            for img in range(NIMG):
                # [0:56, 0:128] vlad accum; [0:56, 128:129] s_k accum (via
                # the appended n-column in the rhs). Sole writer of its bank:
                # any other start=True matmul into this bank would clear it.
                psV = pV.tile([C, 160], f32, tag="psV")
                for ch in range(NCH):
                    xin = xpool.tile([C, CH], f32, tag="x")
                    nc.gpsimd.dma_start(xin[:], x_in[img, :, ch * CH:(ch + 1) * CH])
                    xsq = wpool.tile([C, CH], f32, tag="xsq")
                    nc.vector.tensor_mul(xsq[:], xin[:], xin[:])

                    psumL = pL.tile([C, TPC * K], f32, tag="L")
                    psumT = pT.tile([C, CH], f32, tag="T")
                    psumS = pS.tile([C, TPC], f32, tag="S")
                    for j in range(TPC):
                        xt = xin[:, j * 128:(j + 1) * 128]
                        nc.tensor.matmul(psumL[:, j * K:(j + 1) * K], xt, wT,
                                         start=True, stop=True)
                        nc.tensor.matmul(psumT[:, j * 128:(j + 1) * 128], xt,
                                         ident, start=True, stop=True)
                        nc.tensor.matmul(psumS[:, j:j + 1],
                                         xsq[:, j * 128:(j + 1) * 128], onesc,
                                         start=True, stop=True)

                    ncol = spool.tile([C, TPC], f32, tag="ncol")
                    nc.scalar.activation(ncol[:], psumS[:], Act.Sqrt)
                    invc = spool.tile([C, TPC], f32, tag="invc")
                    nc.vector.reciprocal(invc[:], ncol[:])

                    l3 = lambda t: t[:].rearrange("p (t k) -> p t k", k=K)
                    # u = raw * inv_n  (per-pixel scale, bcast along k)
                    lu = wpool.tile([C, TPC * K], f32, tag="lu")
                    nc.vector.tensor_tensor(
                        l3(lu), l3(psumL),
                        invc[:].broadcast_to([C, TPC, K]), Alu.mult)
                    # l = u + b   (bias per-k, pre-tiled 8x from host)
                    ll = wpool.tile([C, TPC * K], f32, tag="ll")
                    nc.vector.tensor_tensor(ll[:], lu[:], b8, Alu.add)
                    # m = max_k l
                    mcol = spool.tile([C, TPC], f32, tag="mcol")
                    nc.vector.tensor_reduce(mcol[:], l3(ll),
                                            axis=mybir.AxisListType.X,
                                            op=Alu.max)
                    # d = l - m
                    dd = wpool.tile([C, TPC * K], f32, tag="dd")
                    nc.vector.tensor_tensor(
                        l3(dd), l3(ll),
                        mcol[:].broadcast_to([C, TPC, K]), Alu.subtract)
                    # e = exp(d)
                    ee = wpool.tile([C, TPC * K], f32, tag="ee")
                    nc.scalar.activation(ee[:], dd[:], Act.Exp)
                    # sumexp
                    scol = spool.tile([C, TPC], f32, tag="scol")
                    nc.vector.tensor_reduce(scol[:], l3(ee),
                                            axis=mybir.AxisListType.X,
                                            op=Alu.add)
                    gcol = spool.tile([C, TPC], f32, tag="gcol")
                    nc.vector.reciprocal(gcol[:], scol[:])
                    rcol = spool.tile([C, TPC], f32, tag="rcol")
                    nc.vector.tensor_tensor(rcol[:], invc[:], gcol[:], Alu.mult)
                    # a_r = e * (inv_n / sumexp)
                    aa = wpool.tile([C, TPC * K], f32, tag="aa")
                    nc.vector.tensor_tensor(
                        l3(aa), l3(ee),
                        rcol[:].broadcast_to([C, TPC, K]), Alu.mult)
                    # xT evict into [x-tile | n-col] interleaved layout so
                    # each vlad rhs is one contiguous [128, 129] slab
                    xTs = wpool.tile([C, TPC * 129], f32, tag="xTs")
                    xTs_v = xTs[:].rearrange("p (t q) -> p t q", q=129)
                    nc.scalar.activation(
                        xTs_v[:, :, 0:128],
                        psumT[:].rearrange("p (t q) -> p t q", q=128),
                        Act.Copy)
                    nc.vector.tensor_copy(
                        xTs_v[:, :, 128:129],
                        ncol[:].broadcast_to([C, TPC, 1]))

                    if img == 0 and ch == 0:
                        nc.gpsimd.dma_start(dbg_ext[:, 0:TPC * K], aa[:])
                        nc.gpsimd.dma_start(dbg_ext[:, 512:512 + TPC], ncol[:])
                        nc.gpsimd.dma_start(dbg_ext[:, 520:520 + TPC], invc[:])
                        nc.gpsimd.dma_start(dbg_ext[:, 528:528 + TPC], mcol[:])
                        nc.gpsimd.dma_start(dbg_ext[:, 536:536 + TPC], scol[:])

                    # PE observer of the ACT semaphore (xTs write), so each
                    # vlad matmul below carries at most one (DVE) wait.
                    nc.tensor.matmul(psumT[0:1, 0:1], xTs[:, 0:1], onesc,
                                     start=True, stop=True)

                    first = ch == 0
                    last = ch == NCH - 1
                    for j in range(TPC):
                        nc.tensor.matmul(psV[0:KE, 0:129],
                                         aa[:, j * K:j * K + KE],
                                         xTs[:, j * 129:(j + 1) * 129],
                                         start=(first and j == 0),
                                         stop=(last and j == TPC - 1))

                # ---- per-image tail ----
                ps = pF.tile([C, 192], f32, tag="psF")
                negs = spool.tile([KE, 1], f32, tag="negs")
                nc.vector.tensor_scalar_mul(negs[:], psV[0:KE, 128:129], -1.0)
                vk = fpool.tile([KE, C], f32, tag="vk")
                nc.vector.scalar_tensor_tensor(vk[:], cen, negs[:],
                                               psV[0:KE, 0:C],
                                               Alu.mult, Alu.add)
                if img == 0:
                    nc.gpsimd.dma_start(
                        dbg_ext[0:KE, 544:544 + C], vk[:])
                    nc.gpsimd.dma_start(
                        dbg_ext[0:KE, 672:673], negs[:])
                # transpose -> [c, k]
                nc.tensor.matmul(ps[:, 0:KE], vk[:], ident[0:KE, 0:KE],
                                 start=True, stop=True)
                trash = fpool.tile([C, KE], f32, tag="trash")
                ssqk = spool.tile([C, 1], f32, tag="ssqk")
                nc.scalar.activation(trash[:], ps[:, 0:KE], Act.Square,
                                     accum_out=ssqk[:])
                nk = spool.tile([C, 1], f32, tag="nk")
                nc.scalar.activation(nk[:], ssqk[:], Act.Sqrt)
                nkc = spool.tile([C, 1], f32, tag="nkc")
                nc.vector.tensor_scalar_max(nkc[:], nk[:], 1e-12)
                invk = spool.tile([C, 1], f32, tag="invk")
                nc.vector.reciprocal(invk[:], nkc[:])
                t2 = spool.tile([C, 1], f32, tag="t2")
                nc.vector.scalar_tensor_tensor(t2[:], ssqk[:], invk[:], invk[:],
                                               Alu.mult, Alu.mult)
                # scalar matmuls go to a separate bank (start=True clears the
                # whole target bank, and ps[:, 0:KE] is still live)
                tiny = pL.tile([C, TPC * K], f32, tag="L")
                nc.tensor.matmul(tiny[0:1, 0:1], t2[:], onesc,
                                 start=True, stop=True)
                tot = spool.tile([1, 1], f32, tag="tot")
                nc.scalar.activation(tot[:], tiny[0:1, 0:1], Act.Sqrt)
                totc = spool.tile([1, 1], f32, tag="totc")
                nc.vector.tensor_scalar_max(totc[:], tot[:], 1e-12)
                fv = spool.tile([1, 1], f32, tag="fv")
                nc.vector.reciprocal(fv[:], totc[:])
                # broadcast fv to [128,1] via PE (wipes the tiny bank again;
                # tot was already evicted to SBUF)
                nc.tensor.matmul(tiny[:, 2:3], onesr, fv[:],
                                 start=True, stop=True)
                comb = spool.tile([C, 1], f32, tag="comb")
                nc.vector.tensor_tensor(comb[:], invk[:], tiny[:, 2:3], Alu.mult)
                vnT = fpool.tile([C, KE], f32, tag="vnT")
                nc.vector.tensor_scalar(vnT[:], ps[:, 0:KE], comb[:], None,
                                        Alu.mult)
                # transpose back -> [k, c]
                nc.tensor.matmul(ps[0:KE, 64:64 + C], vnT[:], ident,
                                 start=True, stop=True)
                ob = fpool.tile([KE, C], f32, tag="ob")
                nc.scalar.activation(ob[:], ps[0:KE, 64:64 + C], Act.Copy)
                nc.gpsimd.dma_start(out_ext[img], ob[:])

    nc.compile()
    return nc


def _get_nc():
    if "nc" not in _cache:
        _cache["nc"] = _build()
    return _cache["nc"]


def _make_in_maps(inputs):
    x = np.asarray(inputs["x"], dtype=np.float32)
    conv_w = np.asarray(inputs["conv_w"], dtype=np.float32)
    conv_b = np.asarray(inputs["conv_b"], dtype=np.float32)
    centroids = np.asarray(inputs["centroids"], dtype=np.float32)

    N = x.shape[0]
    n_cores = 8
    per = N // n_cores
    assert per == NIMG

    xr = x.reshape(N, C, P)
    cst = np.zeros((C, 960), dtype=np.float32)
    cst[:, 0:K] = conv_w.T
    cst[:, 64:64 + TPC * K] = np.tile(conv_b, TPC)[None, :]
    cst[:, 576:576 + C] = np.eye(C, dtype=np.float32)
    cst[:, 704:832] = 1.0
    cst[0:KE, 832:832 + C] = centroids[:KE]

    in_maps = []
    for i in range(n_cores):
        in_maps.append({
            "x": np.ascontiguousarray(xr[i * per:(i + 1) * per]),
            "consts": cst,
        })
    return in_maps


def kernel(x, conv_w, conv_b, centroids):
    from concourse.bass_utils import run_bass_kernel_spmd

    in_maps = _make_in_maps(
        {"x": x, "conv_w": conv_w, "conv_b": conv_b, "centroids": centroids}
    )
    nc = _get_nc()
    res = run_bass_kernel_spmd(nc, in_maps, list(range(8)))
    outs = [np.asarray(r["out"]).reshape(NIMG, KE * C) for r in res.results]
    return np.concatenate(outs, axis=0)


if __name__ == "__main__":
    rng = np.random.default_rng(0)
    x = rng.standard_normal((32, C, 64, 64), dtype=np.float32)
    w = rng.standard_normal((K, C), dtype=np.float32)
    b = rng.standard_normal((K,), dtype=np.float32)
    c = rng.random((K, C), dtype=np.float32)
    out = kernel(x=x, conv_w=w, conv_b=b, centroids=c)
    print(out.shape, out.dtype)



# revision 12
# speedup vs baseline: 1.4012x; 1.4012x over previous
"""NetVLAD Trainium2 kernel — data-parallel over N across 8 cores.

v2: bf16 PE datapath + fp16 softmax chain + ln/exp-based rsqrt (single
activation table), host-side bf16 upload (halves DMA), merged
logits+transpose matmul, software-pipelined vlad.

Per core: 4 images [C=128, P=4096], chunks of 1024 px (8 tiles of 128).
  PE per tile:  psum[px, 0:64]=logits_raw, [64:192]=xT  via one matmul
                xb_t.T @ [wT | I] (bf16);  ssq via xsqb_t.T @ ones.
  softmax (k in free dim):  inv_n = exp(-.5 ln ssq) [ACT], lu = raw*inv_n
  [DVE fp16], ll = lu + b [DVE fp16], negm = -max_k [DVE], per-tile
  ee = Exp(ll + negm_t) with accum -> sumexp [ACT, bf16 out],
  r = inv_n/sumexp [DVE bf16], aa = ee*r [DVE bf16].
  gpsimd evicts xT psum -> xTs bf16 [px, (8,129)], col 128 = n.
  PE: psV[56, 0:129] += aa_t[:, :56].T @ xTs_t  (bf16, accum over image).
Tail per image in the psV bank: vk = term1 - s*cen, PE transpose,
intra/global norms via Square-accum + ln/exp, transpose back, DMA out.
"""

import sys

for _p in ("/opt/trn_rl_repo",):
    if _p not in sys.path:
        sys.path.insert(0, _p)

import numpy as np

NIMG = 4      # images per core
C = 128
K = 64
KE = 56
P = 4096
TPC = 8       # 128-px tiles per chunk
CH = TPC * 128
NCH = P // CH           # 4 chunks per image
NT = NIMG * NCH         # 16 chunks per core

_cache = {}


def _build():
    import concourse.bass as bass
    import concourse.mybir as mybir
    from concourse import bacc, tile

    f32 = mybir.dt.float32
    f16 = mybir.dt.float16
    bf16 = mybir.dt.bfloat16
    Alu = mybir.AluOpType
    Act = mybir.ActivationFunctionType
    AxX = mybir.AxisListType.X

    nc = bacc.Bacc()
    x_in = nc.declare_dram_parameter("xb", [NIMG, C, P], bf16, isOutput=False)
    # cstb bf16 [C, 193]: 0:64 wT | 64:192 ident | 192 ones
    cb_in = nc.declare_dram_parameter("cstb", [C, 193], bf16, isOutput=False)
    # csth fp32 [C, 512]: conv_b tiled 8x
    ch_in = nc.declare_dram_parameter("csth", [C, 512], f32, isOutput=False)
    # cstf fp32 [C, 392]: 0:128 ident | 128:256 cen(rows 0:56) | 256 ones-col
    # | 258:386 ones-row (row 0)
    cf_in = nc.declare_dram_parameter("cstf", [C, 392], f32, isOutput=False)
    out_ext = nc.declare_dram_parameter("out", [NIMG, KE, C], f32, isOutput=True)

    with tile.TileContext(nc) as tc:
        with (
            tc.tile_pool(name="const", bufs=1) as cpool,
            tc.tile_pool(name="xin", bufs=3) as xpool,
            tc.tile_pool(name="work", bufs=2) as wpool,
            tc.tile_pool(name="stats", bufs=2) as spool,
            tc.tile_pool(name="fin", bufs=2) as fpool,
            tc.tile_pool(name="psL", bufs=2, space="PSUM") as pL,
            tc.tile_pool(name="psT", bufs=2, space="PSUM") as pT,
            tc.tile_pool(name="psS", bufs=1, space="PSUM") as pS,
            tc.tile_pool(name="psV", bufs=1, space="PSUM") as pV,
        ):
            cstb = cpool.tile([C, 193], bf16, tag="cstb")
            csth = cpool.tile([C, 512], f32, tag="csth")
            cstf = cpool.tile([C, 392], f32, tag="cstf")
            nc.sync.dma_start(cstb[:], cb_in[:])
            nc.sync.dma_start(csth[:], ch_in[:])
            nc.sync.dma_start(cstf[:], cf_in[:])
            wTb = cstb[:, 0:K]
            identb = cstb[:, K:K + C]
            onesb = cstb[:, 192:193]
            b8h = csth[:]                 # fp16 bias, tiled 8x
            identf = cstf[:, 0:128]
            cen = cstf[0:KE, 128:256]
            onesf = cstf[:, 256:257]
            onesrow = cstf[0:1, 258:386]

            state = {}

            def emit_chunk(t):
                img, ch = divmod(t, NCH)
                xb = xpool.tile([C, CH], bf16, tag="x")
                nc.sync.dma_start(xb[:], x_in[img, :, ch * CH:(ch + 1) * CH])
                # squares for per-pixel norms (benign precision)
                xsq = wpool.tile([C, CH], bf16, tag="xsq")
                nc.vector.tensor_mul(xsq[:], xb[:], xb[:])

                psl = pL.tile([C, TPC * K], f32, tag="L")
                pst = pT.tile([C, TPC * 128], f32, tag="T")
                pss = pS.tile([C, 8], f32, tag="S")
                for j in range(TPC):
                    xt = xb[:, j * 128:(j + 1) * 128]
                    nc.tensor.matmul(psl[:, j * K:(j + 1) * K], xt, wTb,
                                     start=True, stop=True)
                    nc.tensor.matmul(pst[:, j * 128:(j + 1) * 128], xt,
                                     identb, start=True, stop=True)
                for j in range(TPC):
                    nc.tensor.matmul(pss[:, j:j + 1],
                                     xsq[:, j * 128:(j + 1) * 128], onesb,
                                     start=True, stop=True)

                # deferred from previous chunk: vlad accumulation + tail
                emit_vlad_prev(t)

                psumL = psl[:].rearrange("p (t k) -> p t k", k=K)

                # inv_n = exp(-0.5 ln ssq); n = exp(0.5 ln ssq) (bf16)
                lnq = spool.tile([C, 8], f32, tag="lnq")
                nc.scalar.activation(lnq[:], pss[:], Act.Ln)
                invc = spool.tile([C, 8], f32, tag="invc")
                nc.scalar.activation(invc[:], lnq[:], Act.Exp, scale=-0.5)
                nb = spool.tile([C, 8], bf16, tag="nb")
                nc.scalar.activation(nb[:], lnq[:], Act.Exp, scale=0.5)

                l3 = lambda tl, q: tl[:].rearrange("p (t k) -> p t k", k=q)
                # lu = raw * inv_n  (fp16 out)
                lu = wpool.tile([C, TPC * K], f32, tag="lu")
                nc.vector.tensor_tensor(
                    l3(lu, K), psumL, invc[:].broadcast_to([C, TPC, K]),
                    Alu.mult)
                # ll = lu + b  (gpsimd)
                ll = wpool.tile([C, TPC * K], f32, tag="ll")
                nc.gpsimd.tensor_tensor(ll[:], lu[:], b8h, Alu.add)
                # negm = -max_k ll
                negm = spool.tile([C, 8], f32, tag="negm")
                nc.vector.tensor_reduce(negm[:], l3(ll, K), axis=AxX,
                                        op=Alu.max, negate=True)
                # dd = ll - m
                dd = wpool.tile([C, TPC * K], f32, tag="dd")
                nc.vector.tensor_tensor(
                    l3(dd, K), l3(ll, K), negm[:].broadcast_to([C, TPC, K]),
                    Alu.add)
                # ee = exp(dd) (bf16), sumexp per tile
                ee = wpool.tile([C, TPC * K], bf16, tag="ee")
                nc.scalar.activation(ee[:], dd[:], Act.Exp)
                scol = spool.tile([C, 8], f32, tag="scol")
                nc.vector.tensor_reduce(scol[:], l3(ee, K), axis=AxX,
                                        op=Alu.add)
                # r = inv_n / sumexp (bf16)
                gcol = spool.tile([C, 8], f32, tag="gcol")
                nc.vector.reciprocal(gcol[:], scol[:])
                rcol = spool.tile([C, 8], bf16, tag="rcol")
                nc.vector.tensor_tensor(rcol[:], invc[:], gcol[:], Alu.mult)
                # aa = ee * r  (gpsimd)
                aa = wpool.tile([C, TPC * K], bf16, tag="aa")
                nc.gpsimd.tensor_tensor(
                    l3(aa, K), l3(ee, K), rcol[:].broadcast_to([C, TPC, K]),
                    Alu.mult)
                # evict xT -> [x | n] interleaved slabs (bf16) on ACT.
                # Two copies: a PSUM access pattern may not cross the 2 KB
                # bank boundary (4 tiles of 128 fp32 cols per bank).
                xTs = wpool.tile([C, TPC * 129], bf16, tag="xTs")
                xv = xTs[:].rearrange("p (t q) -> p t q", q=129)
                h = TPC // 2
                pv_ = pst[:].rearrange("p (t q) -> p t q", q=128)
                nc.scalar.activation(xv[:, 0:h, 0:128], pv_[:, 0:h, :],
                                     Act.Copy)
                nc.scalar.activation(xv[:, h:TPC, 0:128], pv_[:, h:TPC, :],
                                     Act.Copy)
                nc.vector.tensor_copy(xv[:, :, 128:129],
                                      nb[:].broadcast_to([C, TPC, 1]))
                state[t] = (aa, xTs)

            def emit_vlad_prev(t):
                tp = t - 1
                if tp < 0 or tp not in state:
                    return
                aa, xTs = state.pop(tp)
                img, ch = divmod(tp, NCH)
                if ch == 0:
                    state["psV"] = pV.tile([C, 512], f32, tag="psV",
                                           name="psv")
                psv = state["psV"]
                for j in range(TPC):
                    nc.tensor.matmul(psv[0:KE, 0:129],
                                     aa[:, j * K:j * K + KE],
                                     xTs[:, j * 129:(j + 1) * 129],
                                     start=(ch == 0 and j == 0),
                                     stop=(ch == NCH - 1 and j == TPC - 1))
                if ch == NCH - 1:
                    emit_tail(img, psv)

            def emit_tail(img, psv):
                # vk = term1 - s*cen  [56, 128] fp32
                negs = spool.tile([KE, 1], f32, tag="negs")
                nc.vector.tensor_scalar_mul(negs[:], psv[0:KE, 128:129], -1.0)
                vk = fpool.tile([KE, C], f32, tag="vk")
                nc.vector.scalar_tensor_tensor(vk[:], cen, negs[:],
                                               psv[0:KE, 0:C],
                                               Alu.mult, Alu.add)
                # transpose -> [c, k] into the same psV bank (cols 160:216)
                nc.tensor.matmul(psv[:, 160:160 + KE], vk[:],
                                 identf[0:KE, 0:KE],
                                 is_transpose=True, start=True, stop=True)
                vkT = psv[:, 160:160 + KE]
                trash = fpool.tile([C, KE], bf16, tag="trash")
                ssqk = spool.tile([C, 1], f32, tag="ssqk")
                nc.scalar.activation(trash[:], vkT, Act.Square,
                                     accum_out=ssqk[:])
                ssqc = spool.tile([C, 1], f32, tag="ssqc")
                nc.vector.tensor_scalar_max(ssqc[:], ssqk[:], 1e-24)
                lnk = spool.tile([C, 1], f32, tag="lnk")
                nc.scalar.activation(lnk[:], ssqc[:], Act.Ln)
                invk = spool.tile([C, 1], f32, tag="invk")
                nc.scalar.activation(invk[:], lnk[:], Act.Exp, scale=-0.5)
                # q = ssqk * invk^2  (per-partition contribution to ||.||_F^2)
                iv2 = spool.tile([C, 1], f32, tag="iv2")
                nc.vector.tensor_tensor(iv2[:], invk[:], invk[:], Alu.mult)
                qv = spool.tile([C, 1], f32, tag="qv")
                nc.vector.tensor_tensor(qv[:], ssqc[:], iv2[:], Alu.mult)
                # tot = sum_c q  via PE (fp32 tiny)
                nc.tensor.matmul(psv[0:1, 216:217], qv[:], onesf,
                                 start=True, stop=True)
                tot = spool.tile([1, 1], f32, tag="tot")
                nc.vector.tensor_scalar_max(tot[:], psv[0:1, 216:217], 1e-24)
                lng = spool.tile([1, 1], f32, tag="lng")
                nc.scalar.activation(lng[:], tot[:], Act.Ln)
                fv = spool.tile([1, 1], f32, tag="fv")
                nc.scalar.activation(fv[:], lng[:], Act.Exp, scale=-0.5)
                # broadcast fv to all partitions, comb = invk * fv
                nc.tensor.matmul(psv[:, 218:219], onesrow, fv[:],
                                 start=True, stop=True)
                comb = spool.tile([C, 1], f32, tag="comb")
                nc.vector.tensor_tensor(comb[:], invk[:], psv[:, 218:219],
                                        Alu.mult)
                obT = fpool.tile([C, KE], f32, tag="obT")
                nc.vector.tensor_scalar(obT[:], vkT, comb[:], None, Alu.mult)
                # transpose back -> [k, c] (cols 256:384), evict, DMA out
                nc.tensor.matmul(psv[0:KE, 256:384], obT[:], identf,
                                 is_transpose=True, start=True, stop=True)
                ob = fpool.tile([KE, C], f32, tag="ob")
                nc.scalar.activation(ob[:], psv[0:KE, 256:384], Act.Copy)
                nc.sync.dma_start(out_ext[img], ob[:])

            for t in range(NT):
                emit_chunk(t)
            emit_vlad_prev(NT)

    nc.compile()
    return nc


def _get_nc():
    if "nc" not in _cache:
        _cache["nc"] = _build()
    return _cache["nc"]


def _make_in_maps(inputs):
    import ml_dtypes

    x = np.asarray(inputs["x"], dtype=np.float32)
    conv_w = np.asarray(inputs["conv_w"], dtype=np.float32)
    conv_b = np.asarray(inputs["conv_b"], dtype=np.float32)
    centroids = np.asarray(inputs["centroids"], dtype=np.float32)

    N = x.shape[0]
    n_cores = 8
    per = N // n_cores
    assert per == NIMG

    xb = x.reshape(N, C, P).astype(ml_dtypes.bfloat16)

    cstb = np.zeros((C, 193), dtype=ml_dtypes.bfloat16)
    cstb[:, 0:K] = conv_w.T.astype(ml_dtypes.bfloat16)
    cstb[:, K:K + C] = np.eye(C, dtype=np.float32)
    cstb[:, 192] = 1.0

    csth = np.tile(conv_b.astype(np.float32), TPC)[None, :].repeat(C, axis=0)
    csth = np.ascontiguousarray(csth)

    cstf = np.zeros((C, 392), dtype=np.float32)
    cstf[:, 0:C] = np.eye(C, dtype=np.float32)
    cstf[0:KE, C:C + C] = centroids[:KE]
    cstf[:, 256] = 1.0
    cstf[0, 258:386] = 1.0

    in_maps = []
    for i in range(n_cores):
        in_maps.append({
            "xb": np.ascontiguousarray(xb[i * per:(i + 1) * per]),
            "cstb": cstb,
            "csth": csth,
            "cstf": cstf,
        })
    return in_maps


def kernel(x, conv_w, conv_b, centroids):
    from concourse.bass_utils import run_bass_kernel_spmd

    in_maps = _make_in_maps(
        {"x": x, "conv_w": conv_w, "conv_b": conv_b, "centroids": centroids}
    )
    nc = _get_nc()
    res = run_bass_kernel_spmd(nc, in_maps, list(range(8)))
    outs = [np.asarray(r["out"]).reshape(NIMG, KE * C) for r in res.results]
    return np.concatenate(outs, axis=0)


if __name__ == "__main__":
    rng = np.random.default_rng(0)
    x = rng.standard_normal((32, C, 64, 64), dtype=np.float32)
    w = rng.standard_normal((K, C), dtype=np.float32)
    b = rng.standard_normal((K,), dtype=np.float32)
    c = rng.random((K, C), dtype=np.float32)
    out = kernel(x=x, conv_w=w, conv_b=b, centroids=c)
    print(out.shape, out.dtype)


# revision 18
# speedup vs baseline: 1.5497x; 1.1060x over previous
"""NetVLAD Trainium2 kernel — data-parallel over N across 8 cores.

v2: bf16 PE datapath + fp16 softmax chain + ln/exp-based rsqrt (single
activation table), host-side bf16 upload (halves DMA), merged
logits+transpose matmul, software-pipelined vlad.

Per core: 4 images [C=128, P=4096], chunks of 1024 px (8 tiles of 128).
  PE per tile:  psum[px, 0:64]=logits_raw, [64:192]=xT  via one matmul
                xb_t.T @ [wT | I] (bf16);  ssq via xsqb_t.T @ ones.
  softmax (k in free dim):  inv_n = exp(-.5 ln ssq) [ACT], lu = raw*inv_n
  [DVE fp16], ll = lu + b [DVE fp16], negm = -max_k [DVE], per-tile
  ee = Exp(ll + negm_t) with accum -> sumexp [ACT, bf16 out],
  r = inv_n/sumexp [DVE bf16], aa = ee*r [DVE bf16].
  gpsimd evicts xT psum -> xTs bf16 [px, (8,129)], col 128 = n.
  PE: psV[56, 0:129] += aa_t[:, :56].T @ xTs_t  (bf16, accum over image).
Tail per image in the psV bank: vk = term1 - s*cen, PE transpose,
intra/global norms via Square-accum + ln/exp, transpose back, DMA out.
"""

import sys

for _p in ("/opt/trn_rl_repo",):
    if _p not in sys.path:
        sys.path.insert(0, _p)

import numpy as np

NIMG = 4      # images per core
C = 128
K = 64
KE = 56
P = 4096
TPC = 8       # 128-px tiles per chunk
CH = TPC * 128
NCH = P // CH           # 4 chunks per image
NT = NIMG * NCH         # 16 chunks per core

_cache = {}


def _build():
    import concourse.bass as bass
    import concourse.mybir as mybir
    from concourse import bacc, tile

    f32 = mybir.dt.float32
    f16 = mybir.dt.float16
    bf16 = mybir.dt.bfloat16
    Alu = mybir.AluOpType
    Act = mybir.ActivationFunctionType
    AxX = mybir.AxisListType.X

    nc = bacc.Bacc()
    x_in = nc.declare_dram_parameter("xb", [NIMG, C, P], bf16, isOutput=False)
    # cstb bf16 [C, 193]: 0:64 wT | 64:192 ident | 192 ones
    cb_in = nc.declare_dram_parameter("cstb", [C, 193], bf16, isOutput=False)
    # csth fp32 [C, 512]: conv_b tiled 8x
    ch_in = nc.declare_dram_parameter("csth", [C, 512], f32, isOutput=False)
    # cstf fp32 [C, 400]: 0:128 ident | 128:256 cen(rows 0:56) | 256 ones-col
    # | 258:386 ones-row (row 0) | 392:400 = -0.5 block
    cf_in = nc.declare_dram_parameter("cstf", [C, 400], f32, isOutput=False)
    out_ext = nc.declare_dram_parameter("out", [NIMG, KE, C], f32, isOutput=True)

    with tile.TileContext(nc) as tc:
        with (
            tc.tile_pool(name="const", bufs=1) as cpool,
            tc.tile_pool(name="xin", bufs=3) as xpool,
            tc.tile_pool(name="work", bufs=2) as wpool,
            tc.tile_pool(name="stats", bufs=2) as spool,
            tc.tile_pool(name="fin", bufs=2) as fpool,
            tc.tile_pool(name="psL", bufs=2, space="PSUM") as pL,
            tc.tile_pool(name="psT", bufs=2, space="PSUM") as pT,
            tc.tile_pool(name="psS", bufs=1, space="PSUM") as pS,
            tc.tile_pool(name="psV", bufs=1, space="PSUM") as pV,
        ):
            cstb = cpool.tile([C, 193], bf16, tag="cstb")
            csth = cpool.tile([C, 512], f32, tag="csth")
            cstf = cpool.tile([C, 400], f32, tag="cstf")
            nc.sync.dma_start(cstb[:], cb_in[:])
            nc.sync.dma_start(csth[:], ch_in[:])
            nc.sync.dma_start(cstf[:], cf_in[:])
            wTb = cstb[:, 0:K]
            identb = cstb[:, K:K + C]
            onesb = cstb[:, 192:193]
            b8h = csth[:]                 # fp16 bias, tiled 8x
            identf = cstf[:, 0:128]
            cen = cstf[0:KE, 128:256]
            onesf = cstf[:, 256:257]
            onesrow = cstf[0:1, 258:386]
            neghalf = cstf[:, 392:400]

            state = {}

            def emit_chunk(t):
                img, ch = divmod(t, NCH)
                xb = xpool.tile([C, CH], bf16, tag="x")
                nc.sync.dma_start(xb[:], x_in[img, :, ch * CH:(ch + 1) * CH])
                # squares for per-pixel norms (benign precision)
                xsq = wpool.tile([C, CH], bf16, tag="xsq")
                nc.scalar.activation(xsq[:], xb[:], Act.Square)

                psl = pL.tile([C, TPC * K], f32, tag="L")
                pst = pT.tile([C, TPC * 128], f32, tag="T")
                pss = pS.tile([C, 8], f32, tag="S")
                for j in range(TPC):
                    xt = xb[:, j * 128:(j + 1) * 128]
                    nc.tensor.matmul(psl[:, j * K:(j + 1) * K], xt, wTb,
                                     start=True, stop=True)
                    nc.tensor.matmul(pst[:, j * 128:(j + 1) * 128], xt,
                                     identb, start=True, stop=True)
                for j in range(TPC):
                    nc.tensor.matmul(pss[:, j:j + 1],
                                     xsq[:, j * 128:(j + 1) * 128], onesb,
                                     start=True, stop=True)

                # deferred from previous chunk: vlad accumulation + tail
                emit_vlad_prev(t)

                psumL = psl[:].rearrange("p (t k) -> p t k", k=K)

                # inv_n = ssq^-0.5 via gpsimd vpowf (keeps ACT on one table)
                ssqs = spool.tile([C, 8], f32, tag="ssqs")
                nc.vector.tensor_copy(ssqs[:], pss[:])
                invc = spool.tile([C, 8], f32, tag="invc")
                nc.gpsimd.tensor_tensor(invc[:], ssqs[:], neghalf, Alu.pow)

                l3 = lambda tl, q: tl[:].rearrange("p (t k) -> p t k", k=q)
                # lu = raw * inv_n  (fp16 out)
                lu = wpool.tile([C, TPC * K], f32, tag="lu")
                nc.vector.tensor_tensor(
                    l3(lu, K), psumL, invc[:].broadcast_to([C, TPC, K]),
                    Alu.mult)
                # ll = lu + b  (gpsimd)
                ll = wpool.tile([C, TPC * K], f32, tag="ll")
                nc.gpsimd.tensor_tensor(ll[:], lu[:], b8h, Alu.add)
                # negm = -max_k ll
                negm = spool.tile([C, 8], f32, tag="negm")
                nc.vector.tensor_reduce(negm[:], l3(ll, K), axis=AxX,
                                        op=Alu.max, negate=True)
                # dd = ll - m
                dd = wpool.tile([C, TPC * K], f32, tag="dd")
                nc.vector.tensor_tensor(
                    l3(dd, K), l3(ll, K), negm[:].broadcast_to([C, TPC, K]),
                    Alu.add)
                # ee = exp(dd) (bf16), sumexp per tile
                ee = wpool.tile([C, TPC * K], bf16, tag="ee")
                nc.scalar.activation(ee[:], dd[:], Act.Exp)
                scol = spool.tile([C, 8], f32, tag="scol")
                nc.vector.tensor_reduce(scol[:], l3(ee, K), axis=AxX,
                                        op=Alu.add)
                # r = inv_n / sumexp (bf16)
                gcol = spool.tile([C, 8], f32, tag="gcol")
                nc.vector.reciprocal(gcol[:], scol[:])
                rcol = spool.tile([C, 8], bf16, tag="rcol")
                nc.vector.tensor_tensor(rcol[:], invc[:], gcol[:], Alu.mult)
                # aa = ee * r  (gpsimd)
                aa = wpool.tile([C, TPC * K], bf16, tag="aa")
                nc.gpsimd.tensor_tensor(
                    l3(aa, K), l3(ee, K), rcol[:].broadcast_to([C, TPC, K]),
                    Alu.mult)
                # evict xT -> [x | n] interleaved slabs (bf16) on ACT.
                # Two copies: a PSUM access pattern may not cross the 2 KB
                # bank boundary (4 tiles of 128 fp32 cols per bank).
                xTs = wpool.tile([C, TPC * 129], bf16, tag="xTs")
                xv = xTs[:].rearrange("p (t q) -> p t q", q=129)
                h = TPC // 2
                pv_ = pst[:].rearrange("p (t q) -> p t q", q=128)
                nc.scalar.activation(xv[:, 0:h, 0:128], pv_[:, 0:h, :],
                                     Act.Copy)
                nc.scalar.activation(xv[:, h:TPC, 0:128], pv_[:, h:TPC, :],
                                     Act.Copy)
                # n = ssq * inv_n written straight into the 129th column
                s3 = lambda tl: tl[:].rearrange("p (t o) -> p t o", o=1)
                nc.gpsimd.tensor_tensor(xv[:, :, 128:129], s3(ssqs),
                                        s3(invc), Alu.mult)
                state[t] = (aa, xTs)

            def emit_vlad_prev(t):
                tp = t - 1
                if tp < 0 or tp not in state:
                    return
                aa, xTs = state.pop(tp)
                img, ch = divmod(tp, NCH)
                if ch == 0:
                    state["psV"] = pV.tile([C, 512], f32, tag="psV",
                                           name="psv")
                psv = state["psV"]
                for j in range(TPC):
                    nc.tensor.matmul(psv[0:KE, 0:129],
                                     aa[:, j * K:j * K + KE],
                                     xTs[:, j * 129:(j + 1) * 129],
                                     start=(ch == 0 and j == 0),
                                     stop=(ch == NCH - 1 and j == TPC - 1))
                if ch == NCH - 1:
                    emit_tail(img, psv)

            def emit_tail(img, psv):
                # vk = term1 - s*cen  [56, 128] fp32
                negs = spool.tile([KE, 1], f32, tag="negs")
                nc.vector.tensor_scalar_mul(negs[:], psv[0:KE, 128:129], -1.0)
                vk = fpool.tile([KE, C], f32, tag="vk")
                nc.vector.scalar_tensor_tensor(vk[:], cen, negs[:],
                                               psv[0:KE, 0:C],
                                               Alu.mult, Alu.add)
                # transpose -> [c, k] into the same psV bank (cols 160:216)
                nc.tensor.matmul(psv[:, 160:160 + KE], vk[:],
                                 identf[0:KE, 0:KE],
                                 is_transpose=True, start=True, stop=True)
                vkT = psv[:, 160:160 + KE]
                trash = fpool.tile([C, KE], bf16, tag="trash")
                ssqk = spool.tile([C, 1], f32, tag="ssqk")
                nc.scalar.activation(trash[:], vkT, Act.Square,
                                     accum_out=ssqk[:])
                ssqc = spool.tile([C, 1], f32, tag="ssqc")
                nc.vector.tensor_scalar_max(ssqc[:], ssqk[:], 1e-24)
                invk = spool.tile([C, 1], f32, tag="invk")
                nc.gpsimd.tensor_tensor(invk[:], ssqc[:], neghalf[:, 0:1],
                                        Alu.pow)
                # q = ssqk * invk^2  (per-partition contribution to ||.||_F^2)
                iv2 = spool.tile([C, 1], f32, tag="iv2")
                nc.vector.tensor_tensor(iv2[:], invk[:], invk[:], Alu.mult)
                qv = spool.tile([C, 1], f32, tag="qv")
                nc.vector.tensor_tensor(qv[:], ssqc[:], iv2[:], Alu.mult)
                # tot = sum_c q  via PE (fp32 tiny)
                nc.tensor.matmul(psv[0:1, 216:217], qv[:], onesf,
                                 start=True, stop=True)
                tot = spool.tile([1, 1], f32, tag="tot")
                nc.vector.tensor_scalar_max(tot[:], psv[0:1, 216:217], 1e-24)
                fv = spool.tile([1, 1], f32, tag="fv")
                nc.gpsimd.tensor_tensor(fv[:], tot[:], neghalf[0:1, 0:1],
                                        Alu.pow)
                # broadcast fv to all partitions, comb = invk * fv
                nc.tensor.matmul(psv[:, 218:219], onesrow, fv[:],
                                 start=True, stop=True)
                comb = spool.tile([C, 1], f32, tag="comb")
                nc.vector.tensor_tensor(comb[:], invk[:], psv[:, 218:219],
                                        Alu.mult)
                obT = fpool.tile([C, KE], f32, tag="obT")
                nc.vector.tensor_scalar(obT[:], vkT, comb[:], None, Alu.mult)
                # transpose back -> [k, c] (cols 256:384), evict, DMA out
                nc.tensor.matmul(psv[0:KE, 256:384], obT[:], identf,
                                 is_transpose=True, start=True, stop=True)
                ob = fpool.tile([KE, C], f32, tag="ob")
                nc.scalar.activation(ob[:], psv[0:KE, 256:384], Act.Copy)
                nc.sync.dma_start(out_ext[img], ob[:])

            for t in range(NT):
                emit_chunk(t)
            emit_vlad_prev(NT)

    nc.compile()
    return nc


def _get_nc():
    if "nc" not in _cache:
        _cache["nc"] = _build()
    return _cache["nc"]


def _make_in_maps(inputs):
    import ml_dtypes

    x = np.asarray(inputs["x"], dtype=np.float32)
    conv_w = np.asarray(inputs["conv_w"], dtype=np.float32)
    conv_b = np.asarray(inputs["conv_b"], dtype=np.float32)
    centroids = np.asarray(inputs["centroids"], dtype=np.float32)

    N = x.shape[0]
    n_cores = 8
    per = N // n_cores
    assert per == NIMG

    xb = x.reshape(N, C, P).astype(ml_dtypes.bfloat16)

    cstb = np.zeros((C, 193), dtype=ml_dtypes.bfloat16)
    cstb[:, 0:K] = conv_w.T.astype(ml_dtypes.bfloat16)
    cstb[:, K:K + C] = np.eye(C, dtype=np.float32)
    cstb[:, 192] = 1.0

    csth = np.tile(conv_b.astype(np.float32), TPC)[None, :].repeat(C, axis=0)
    csth = np.ascontiguousarray(csth)

    cstf = np.zeros((C, 400), dtype=np.float32)
    cstf[:, 0:C] = np.eye(C, dtype=np.float32)
    cstf[0:KE, C:C + C] = centroids[:KE]
    cstf[:, 256] = 1.0
    cstf[0, 258:386] = 1.0
    cstf[:, 392:400] = -0.5

    in_maps = []
    for i in range(n_cores):
        in_maps.append({
            "xb": np.ascontiguousarray(xb[i * per:(i + 1) * per]),
            "cstb": cstb,
            "csth": csth,
            "cstf": cstf,
        })
    return in_maps


def kernel(x, conv_w, conv_b, centroids):
    from concourse.bass_utils import run_bass_kernel_spmd

    in_maps = _make_in_maps(
        {"x": x, "conv_w": conv_w, "conv_b": conv_b, "centroids": centroids}
    )
    nc = _get_nc()
    res = run_bass_kernel_spmd(nc, in_maps, list(range(8)))
    outs = [np.asarray(r["out"]).reshape(NIMG, KE * C) for r in res.results]
    return np.concatenate(outs, axis=0)


if __name__ == "__main__":
    rng = np.random.default_rng(0)
    x = rng.standard_normal((32, C, 64, 64), dtype=np.float32)
    w = rng.standard_normal((K, C), dtype=np.float32)
    b = rng.standard_normal((K,), dtype=np.float32)
    c = rng.random((K, C), dtype=np.float32)
    out = kernel(x=x, conv_w=w, conv_b=b, centroids=c)
    print(out.shape, out.dtype)
